# revision 16
# baseline (speedup 1.0000x reference)
"""GATv2 3-layer encoder on 8 Trainium2 NeuronCores (Bass/Tile).

Sharding: nodes split contiguously across 8 cores (graph parallel). Edges
(with self-loops) are owned by the dst node's core; per core they are
grouped by (dst block, src chunk) with per-(block,chunk) runs padded to
128-edge tiles. Per layer: local matmuls (HL|HR = h @ [Wl|Wr]), AllGather
of the HL shard into a full [N,128] table, then an edge phase:
  - hl[src] gathered per (block, chunk) by ONE hardware dma_gather
    (InstDMAGatherAnt, int16 chunk-relative indices, ~5-8 tiles per
    instruction) - amortizes the ~1us fixed SWDGE overhead that dominated
    the per-tile-indirect-DMA baseline. The full-node table is split into
    4 chunks of 25000 rows so indices fit int16.
  - one-hot selector matrices S_e ([edge,dst]) and S_eT ([dst,edge]) are
    precomputed on host (graph constants, reused by all 3 layers) and
    streamed from DRAM per dst block - no on-chip selector builds,
  - tt = G^T + hr^T S_eT accumulated in PSUM: 1 hr matmul per 4-tile
    subgroup + 1 transpose matmul per tile; LeakyReLU on ScalarE (one op
    per subgroup),
  - per-tile score matmul (lhsT = t2t tile, rhs = block-diag attention)
    yields scores in [edge, head] layout; Exp writes exp(score) directly
    into the seg-matmul RHS weight columns (no PSUM->SBUF transposes),
  - per-edge alpha-weighting as one DVE multiply per tile,
  - segment softmax denominator + weighted feature sum via one PE matmul
    per tile (lhsT = S_e), accumulating per 128-dst block in PSUM,
  - per-block epilogue: normalize, bias, relu (f32), fused next-layer
    node-phase matmuls.

kernel(**inputs) takes FULL inputs, returns the FULL [100000, 64] f32 output.
"""

import math
from contextlib import ExitStack

import numpy as np
import ml_dtypes

import concourse.bass as bass
import concourse.tile as tile
from concourse import bacc, mybir
from concourse.bass_utils import run_bass_kernel_spmd

# -------- problem config (hardcoded; must match reference.setup_inputs) ----
N_NODES = 100_000
N_EDGES = 1_600_000
NCORES = 8
NEG_SLOPE = 0.2
P = 128
LAYERS = [(128, 128, 4), (128, 128, 4), (128, 64, 1)]  # (D_in, D_out, heads)
PAD_OFF = 200.0
EPS = 1e-16
GRP = 4       # tiles per matmul/prelu subgroup (PSUM-bank limited)
NCHUNK = 4    # gather-table chunks (int16 index range)
CH_ROWS = N_NODES // NCHUNK

f32 = mybir.dt.float32
bf16 = mybir.dt.bfloat16
i16 = mybir.dt.int16

_CACHE = {}
_LAST_RES = None


# ---------------------------------------------------------------------------
# host-side preprocessing
# ---------------------------------------------------------------------------
def _balance_perm(dst, n, nloc, nblk):
    """Assign nodes to (core, block) bins balancing per-bin edge counts.
    Returns pos_of[old_id] -> new global position."""
    import heapq
    deg = np.bincount(dst, minlength=n).astype(np.int64)
    caps = np.zeros((NCORES, nblk), np.int64)
    caps[:, :] = P
    last = nloc - (nblk - 1) * P
    caps[:, nblk - 1] = last
    order = np.argsort(-deg, kind="stable")
    heap = [(0, c * nblk + b) for c in range(NCORES) for b in range(nblk)]
    heapq.heapify(heap)
    fill = np.zeros(NCORES * nblk, np.int64)
    pos_of = np.empty(n, np.int64)
    for v in order:
        while True:
            w, bin_ = heapq.heappop(heap)
            c, b = divmod(bin_, nblk)
            if fill[bin_] < caps[c, b]:
                break
        s = fill[bin_]
        fill[bin_] += 1
        pos_of[v] = c * nloc + b * P + s
        if fill[bin_] < caps[c, b]:
            heapq.heappush(heap, (w + deg[v], bin_))
    return pos_of


def _preprocess(edge_index):
    n = N_NODES
    nloc = n // NCORES
    nblk = math.ceil(nloc / P)

    src = np.concatenate([edge_index[0], np.arange(n, dtype=np.int64)]).astype(np.int64)
    dst = np.concatenate([edge_index[1], np.arange(n, dtype=np.int64)]).astype(np.int64)

    pos_of = _balance_perm(dst, n, nloc, nblk)
    src = pos_of[src]
    dst = pos_of[dst]

    core_of = dst // nloc
    blk_of = (dst % nloc) // P
    ch_of = src // CH_ROWS
    counts4 = np.zeros((NCORES, nblk, NCHUNK), np.int64)
    np.add.at(counts4, (core_of, blk_of, ch_of), 1)
    tiles_bch = np.ceil(counts4.max(axis=0) / P).astype(np.int64)  # [nblk, 4]
    tbp = tiles_bch.sum(axis=1)                                    # [nblk]
    t_total = int(tbp.sum())
    gcol = np.concatenate([[0], np.cumsum(tbp)])[:-1]
    ch_toff = np.concatenate(
        [np.zeros((nblk, 1), np.int64), np.cumsum(tiles_bch, axis=1)],
        axis=1)[:, :NCHUNK]

    jar = np.arange(P, dtype=np.float32)
    per_core = []
    for c in range(NCORES):
        eids = np.where(core_of == c)[0]
        s_c = src[eids]
        d_c = dst[eids] - c * nloc
        b_c = d_c // P
        off_c = (d_c % P).astype(np.float32)
        chc = s_c // CH_ROWS
        order = np.lexsort((off_c, chc, b_c))
        s_c, off_c, chc, b_c = s_c[order], off_c[order], chc[order], b_c[order]

        key = b_c * NCHUNK + chc
        starts = np.searchsorted(key, np.arange(nblk * NCHUNK))
        ends = np.searchsorted(key, np.arange(nblk * NCHUNK) + 1)
        flat_idx = np.zeros(t_total * P, np.int16)
        dstoff = np.full((t_total, P), PAD_OFF, np.float32)
        for b in range(nblk):
            for ch in range(NCHUNK):
                e0, e1 = starts[b * NCHUNK + ch], ends[b * NCHUNK + ch]
                cnt = e1 - e0
                if cnt == 0:
                    continue
                flat0 = (gcol[b] + ch_toff[b, ch]) * P
                flat_idx[flat0:flat0 + cnt] = \
                    (s_c[e0:e1] - ch * CH_ROWS).astype(np.int16)
                dstoff.reshape(-1)[flat0:flat0 + cnt] = off_c[e0:e1]
        # idx layout for dma_gather: entry i at [i%16, i//16], tiled to 128
        idx16 = np.tile(np.ascontiguousarray(flat_idx.reshape(-1, 16).T),
                        (8, 1))

        # one-hot selectors: per block [SeT tiles | Se tiles] (bf16)
        se_all = (dstoff[:, :, None] == jar[None, None, :])
        se_flat = np.ascontiguousarray(
            se_all.transpose(1, 0, 2).reshape(P, t_total * P))
        seT_flat = np.ascontiguousarray(
            se_all.transpose(2, 0, 1).reshape(P, t_total * P))
        sel = np.empty((P, t_total * 2 * P), np.float32)
        for b in range(nblk):
            c0 = int(gcol[b])
            tb = int(tbp[b])
            base = c0 * 2 * P
            sel[:, base:base + tb * P] = seT_flat[:, c0 * P:(c0 + tb) * P]
            sel[:, base + tb * P:base + 2 * tb * P] = \
                se_flat[:, c0 * P:(c0 + tb) * P]

        per_core.append({
            "idx16": idx16,                            # [128, t_total*8] i16
            "sel": sel.astype(ml_dtypes.bfloat16),     # [128, t_total*256]
        })
    meta = (tiles_bch, tbp, gcol, ch_toff, t_total)
    return meta, per_core, pos_of


def _host_consts(inputs):
    c = {}
    for li, (din, dout, h) in enumerate(LAYERS):
        wl = np.asarray(inputs[f"W{li}l"], np.float32)
        wr = np.asarray(inputs[f"W{li}r"], np.float32)
        att = np.asarray(inputs[f"a{li}"], np.float32)
        bias = np.asarray(inputs[f"b{li}"], np.float32)
        ch = dout // h
        a_bd = np.zeros((dout, h), np.float32)
        for hh in range(h):
            a_bd[hh * ch:(hh + 1) * ch, hh] = att[hh]
        c[f"w2_{li}"] = np.concatenate([wl, wr], axis=1).astype(ml_dtypes.bfloat16)
        c[f"abd_{li}"] = a_bd.astype(ml_dtypes.bfloat16)
        c[f"bias_{li}"] = np.ascontiguousarray(np.tile(bias[None, :], (P, 1)))
    c["ident"] = np.eye(P, dtype=ml_dtypes.bfloat16)
    return c


# ---------------------------------------------------------------------------
# device program
# ---------------------------------------------------------------------------
def _build_program(meta, nloc):
    tiles_bch, tbp, gcol, ch_toff, t_total = meta
    nblk = len(tbp)
    tbpmax = int(tbp.max())
    nlocp = nblk * P
    n = nloc * NCORES

    nc = bacc.Bacc("TRN2", target_bir_lowering=False, debug=False,
                   num_devices=NCORES, dynamic_dma_scratch_size=32768,
                   num_swdge_queues=4)

    x_local = nc.dram_tensor("x_local", [nlocp, 128], bf16, kind="ExternalInput")
    idx_in = nc.dram_tensor("idx16", [P, t_total * 8], i16, kind="ExternalInput")
    sel_in = nc.dram_tensor("sel", [P, t_total * 2 * P], bf16,
                            kind="ExternalInput")
    ident_in = nc.dram_tensor("ident", [P, P], bf16, kind="ExternalInput")
    w2_in, abd_in, bias_in = {}, {}, {}
    for li, (din, dout, h) in enumerate(LAYERS):
        w2_in[li] = nc.dram_tensor(f"w2_{li}", [din, 2 * dout], bf16,
                                   kind="ExternalInput")
        abd_in[li] = nc.dram_tensor(f"abd_{li}", [dout, h], bf16,
                                    kind="ExternalInput")
        bias_in[li] = nc.dram_tensor(f"bias_{li}", [P, dout], f32,
                                     kind="ExternalInput")
    out_t = nc.dram_tensor("out", [nloc, 64], f32, kind="ExternalOutput")

    with tile.TileContext(nc) as tc, ExitStack() as ctx:
        cn = ctx.enter_context(tc.tile_pool(name="cn", bufs=1))
        dr = ctx.enter_context(tc.tile_pool(name="dr", bufs=1, space="DRAM"))

        hr_buf = {0: dr.tile([nlocp + P, 128], bf16, tag="hr0", name="hr0")}
        hr_buf[1] = dr.tile([nlocp + P, 128], bf16, tag="hr1", name="hr1")
        hr_buf[2] = dr.tile([nlocp + P, 64], bf16, tag="hr2", name="hr2")
        # bounce/hlf tables are 128 cols for every layer (layer-2 cols 64:
        # are zero) so the gather elem_size is always 128 (256B rows)
        bounce = {li: dr.tile([nloc, 128], bf16, tag=f"bnc{li}",
                              name=f"bnc{li}") for li in range(3)}
        hlf = {li: dr.tile([n, 128], bf16, addr_space="Shared",
                           tag=f"hlf{li}", name=f"hlf{li}") for li in range(3)}

        ident = cn.tile([P, P], bf16)
        nc.sync.dma_start(out=ident[:], in_=ident_in[:, :])
        idx16 = cn.tile([P, t_total * 8], i16)
        nc.sync.dma_start(out=idx16[:], in_=idx_in[:, :])
        w2_sb, abd_sb, bias_sb = {}, {}, {}
        for li, (din, dout, h) in enumerate(LAYERS):
            w2_sb[li] = cn.tile([din, 2 * dout], bf16, tag=f"w2s{li}",
                                name=f"w2s{li}")
            nc.sync.dma_start(out=w2_sb[li][:], in_=w2_in[li][:, :])
            abd_sb[li] = cn.tile([dout, h], bf16, tag=f"abds{li}",
                                 name=f"abds{li}")
            nc.sync.dma_start(out=abd_sb[li][:], in_=abd_in[li][:, :])
            bias_sb[li] = cn.tile([P, dout], f32, tag=f"biass{li}",
                                  name=f"biass{li}")
            nc.sync.dma_start(out=bias_sb[li][:], in_=bias_in[li][:, :])

        zpad = cn.tile([P, 128], bf16)
        nc.vector.memset(zpad[:], 0.0)
        for r0 in range(nloc, nlocp + P, P):
            rows = min(P, nlocp + P - r0)
            nc.sync.dma_start(out=hr_buf[0][r0:r0 + rows, :], in_=zpad[:rows, :])
            nc.sync.dma_start(out=hr_buf[1][r0:r0 + rows, :], in_=zpad[:rows, :])
            nc.sync.dma_start(out=hr_buf[2][r0:r0 + rows, :64],
                              in_=zpad[:rows, :64])
        # layer-2 bounce cols 64:128 stay zero forever
        for r0 in range(0, nloc, P):
            rows = min(P, nloc - r0)
            nc.sync.dma_start(out=bounce[2][r0:r0 + rows, 64:128],
                              in_=zpad[:rows, :64])

        # ================= layers =================
        ett = ctx.enter_context(tc.tile_pool(name="ett", bufs=2, space="PSUM"))
        esc = ctx.enter_context(tc.tile_pool(name="esc", bufs=2, space="PSUM"))
        enp = ctx.enter_context(tc.tile_pool(name="enp", bufs=1, space="PSUM"))
        eu = ctx.enter_context(tc.tile_pool(name="eu", bufs=2, space="PSUM"))
        esb = ctx.enter_context(tc.tile_pool(name="esb", bufs=2))
        blkp = ctx.enter_context(tc.tile_pool(name="blkp", bufs=4))
        nsb = ctx.enter_context(tc.tile_pool(name="nsb", bufs=4))
        for li, (din, dout, h) in enumerate(LAYERS):
            ch_ = dout // h
            dh = dout + h

            # ---- node phase (standalone for layer 0 only; later layers are
            # fused into the previous edge phase epilogue) ----
            if li == 0:
                for nt in range(nblk):
                    r0 = nt * P
                    rows = min(P, nloc - r0)
                    x_sb = nsb.tile([P, din], bf16, tag="x")
                    nc.sync.dma_start(out=x_sb[:], in_=x_local[r0:r0 + P, :din])
                    xT_ps = enp.tile([P, P], f32, tag="nps")
                    nc.tensor.matmul(out=xT_ps[:din, :P], lhsT=x_sb[:],
                                     rhs=ident[:], start=True, stop=True)
                    xT = nsb.tile([P, P], bf16, tag="xTs")
                    nc.vector.tensor_copy(out=xT[:din, :], in_=xT_ps[:din, :P])
                    hlr_ps = enp.tile([P, 2 * dout], f32, tag="nps",
                                      name="hlrps")
                    nc.tensor.matmul(out=hlr_ps[:], lhsT=xT[:din, :],
                                     rhs=w2_sb[li][:], start=True, stop=True)
                    hl_sb = nsb.tile([P, dout], bf16, tag="hl")
                    nc.scalar.activation(out=hl_sb[:], in_=hlr_ps[:, 0:dout],
                                         func=mybir.ActivationFunctionType.Copy)
                    hr_sb = nsb.tile([P, dout], bf16, tag="hr")
                    nc.scalar.activation(out=hr_sb[:], in_=hlr_ps[:, dout:],
                                         func=mybir.ActivationFunctionType.Copy)
                    nc.sync.dma_start(out=bounce[li][r0:r0 + rows, :dout],
                                      in_=hl_sb[:rows, :])
                    nc.sync.dma_start(out=hr_buf[li][r0:r0 + rows, :dout],
                                      in_=hr_sb[:rows, :])

                nc.gpsimd.collective_compute(
                    "AllGather", mybir.AluOpType.bypass,
                    replica_groups=[list(range(NCORES))],
                    ins=[bounce[0][:].opt()], outs=[hlf[0][:].opt()])

            # ---- edge phase ----
            for b in range(nblk):
                r0 = b * P
                rows = min(P, nloc - r0)
                tb = int(tbp[b])
                c0 = int(gcol[b])
                hrb = blkp.tile([P, dout], bf16, tag="hrb")
                nc.sync.dma_start(out=hrb[:],
                                  in_=hr_buf[li][r0:r0 + P, :dout])
                # selector block: [SeT tiles (tb*P) | Se tiles (tb*P)]
                sel_blk = blkp.tile([P, tbpmax * 2 * P], bf16, tag="sel")
                nc.sync.dma_start(
                    out=sel_blk[:, :tb * 2 * P],
                    in_=sel_in[:, c0 * 2 * P:(c0 + tb) * 2 * P])
                # gathered hl[src] for all tiles of the block, one dma_gather
                # per source chunk (tile-aligned, independent)
                G_blk = blkp.tile([P, tbpmax * P], bf16, tag="G")
                for ch in range(NCHUNK):
                    ntile = int(tiles_bch[b, ch])
                    if ntile == 0:
                        continue
                    tofs = int(ch_toff[b, ch])
                    nidx = ntile * P
                    icol = (c0 + tofs) * 8
                    nc.gpsimd.dma_gather(
                        G_blk[:, (tofs) * P:(tofs + ntile) * P].rearrange(
                            "p (t c) -> p t c", c=P),
                        hlf[li][ch * CH_ROWS:(ch + 1) * CH_ROWS, :],
                        idx16[:, icol:icol + ntile * 8],
                        nidx, nidx, P, queue_num=(b * NCHUNK + ch) % 4)
                u_ps = eu.tile([P, dh], f32, tag="u")

                for t0 in range(0, tb, GRP):
                    k = min(GRP, tb - t0)
                    rhs_blk = esb.tile([P, GRP * dh], bf16, tag="rhs")
                    # tt = hr^T @ SeT (one mm) + per-tile G^T transposes
                    tt_ps = ett.tile([P, GRP * P], f32, tag="tt")
                    nc.tensor.matmul(out=tt_ps[:dout, :k * P],
                                     lhsT=hrb[:, :dout],
                                     rhs=sel_blk[:, t0 * P:(t0 + k) * P],
                                     start=True, stop=False,
                                     skip_group_check=True)
                    for gi in range(k):
                        nc.tensor.matmul(
                            out=tt_ps[:dout, gi * P:(gi + 1) * P],
                            lhsT=G_blk[:, (t0 + gi) * P:(t0 + gi) * P + dout],
                            rhs=ident[:], start=False, stop=True,
                            skip_group_check=True)
                    t2t = esb.tile([P, GRP * P], bf16, tag="t2t")
                    nc.scalar.activation(
                        out=t2t[:dout, :k * P], in_=tt_ps[:dout, :k * P],
                        func=mybir.ActivationFunctionType.Prelu,
                        alpha=NEG_SLOPE)
                    # per-tile scores in [edge, head] layout
                    sc_ps = esc.tile([P, GRP * 4], f32, tag="sc")
                    for gi in range(k):
                        nc.tensor.matmul(
                            out=sc_ps[:, gi * h:(gi + 1) * h],
                            lhsT=t2t[:dout, gi * P:(gi + 1) * P],
                            rhs=abd_sb[li][:], start=True, stop=True)
                    rview = rhs_blk[:, :k * dh].rearrange(
                        "p (t c) -> p t c", c=dh)
                    # exp(score) into the weight cols of rhs_blk (one op)
                    nc.scalar.activation(
                        out=rview[:, :, dout:dh],
                        in_=sc_ps[:, :k * h].rearrange("p (t h) -> p t h",
                                                       h=h),
                        func=mybir.ActivationFunctionType.Exp)
                    # alpha-weighted features into rhs_blk (one op)
                    nc.vector.tensor_tensor(
                        out=rview[:, :, 0:dout].rearrange(
                            "p t (h c) -> p t h c", h=h),
                        in0=G_blk[:, t0 * P:t0 * P + k * P].rearrange(
                            "p (t c) -> p t c", c=P)[:, :, 0:dout].rearrange(
                            "p t (h c) -> p t h c", h=h),
                        in1=rview[:, :, dout:dh].to_broadcast([P, k, h, ch_]),
                        op=mybir.AluOpType.mult)
                    # per-tile segment sum (features + denominator)
                    for gi in range(k):
                        t = t0 + gi
                        nc.tensor.matmul(
                            out=u_ps[:],
                            lhsT=sel_blk[:, (tb + t) * P:(tb + t + 1) * P],
                            rhs=rhs_blk[:, gi * dh:(gi + 1) * dh],
                            start=(t == 0), stop=(t == tb - 1))

                # ---- block epilogue (f32) ----
                den = esb.tile([P, h], f32, tag="den")
                nc.vector.tensor_scalar(
                    out=den[:], in0=u_ps[:, dout:dh], scalar1=EPS,
                    scalar2=None, op0=mybir.AluOpType.add)
                rden = esb.tile([P, h], f32, tag="rden")
                nc.vector.reciprocal(out=rden[:], in_=den[:])
                o_sb = esb.tile([P, dout], f32, tag="osb")
                if h > 1:
                    nc.vector.tensor_tensor(
                        out=o_sb[:].rearrange("p (h c) -> p h c", h=h),
                        in0=u_ps[:, 0:dout].rearrange("p (h c) -> p h c",
                                                      h=h),
                        in1=rden[:].to_broadcast([P, h, ch_]),
                        op=mybir.AluOpType.mult)
                else:
                    nc.vector.tensor_scalar(
                        out=o_sb[:], in0=u_ps[:, 0:dout],
                        scalar1=rden[:, 0:1], scalar2=None,
                        op0=mybir.AluOpType.mult)
                nc.vector.tensor_tensor(out=o_sb[:], in0=o_sb[:],
                                        in1=bias_sb[li][:, :dout],
                                        op=mybir.AluOpType.add)
                if li < 2:
                    o2_sb = esb.tile([P, dout], bf16, tag="o2sb")
                    nc.scalar.activation(
                        out=o2_sb[:], in_=o_sb[:],
                        func=mybir.ActivationFunctionType.Relu)
                    # fused node phase for layer li+1, block b
                    din2 = LAYERS[li + 1][0]
                    dout2 = LAYERS[li + 1][1]
                    xT_ps = enp.tile([P, P], f32, tag="nps", name="xTf")
                    nc.tensor.matmul(out=xT_ps[:din2, :], lhsT=o2_sb[:],
                                     rhs=ident[:], start=True, stop=True)
                    xTf = nsb.tile([P, P], bf16, tag="xTs", name="xTfs")
                    nc.vector.tensor_copy(out=xTf[:din2, :],
                                          in_=xT_ps[:din2, :])
                    hlr2 = enp.tile([P, 2 * dout2], f32, tag="nps",
                                    name="hlr2")
                    nc.tensor.matmul(out=hlr2[:], lhsT=xTf[:din2, :],
                                     rhs=w2_sb[li + 1][:], start=True,
                                     stop=True)
                    hl2 = nsb.tile([P, dout2], bf16, tag="hl", name="hl2")
                    nc.scalar.activation(
                        out=hl2[:], in_=hlr2[:, 0:dout2],
                        func=mybir.ActivationFunctionType.Copy)
                    hr2s = nsb.tile([P, dout2], bf16, tag="hr", name="hr2s")
                    nc.scalar.activation(
                        out=hr2s[:], in_=hlr2[:, dout2:],
                        func=mybir.ActivationFunctionType.Copy)
                    nc.sync.dma_start(out=bounce[li + 1][r0:r0 + rows, :dout2],
                                      in_=hl2[:rows, :])
                    nc.sync.dma_start(
                        out=hr_buf[li + 1][r0:r0 + rows, :dout2],
                        in_=hr2s[:rows, :])
                else:
                    o2f = esb.tile([P, dout], f32, tag="o2f")
                    nc.scalar.activation(
                        out=o2f[:], in_=o_sb[:],
                        func=mybir.ActivationFunctionType.Relu)
                    nc.sync.dma_start(out=out_t[r0:r0 + rows, :],
                                      in_=o2f[:rows, :])

            if li < 2:
                nc.gpsimd.collective_compute(
                    "AllGather", mybir.AluOpType.bypass,
                    replica_groups=[list(range(NCORES))],
                    ins=[bounce[li + 1][:].opt()],
                    outs=[hlf[li + 1][:].opt()])

    nc.compile()
    return nc


def _run(inputs, trace=False):
    n = N_NODES
    nloc = n // NCORES
    nblk = math.ceil(nloc / P)
    nlocp = nblk * P

    if "prog" not in _CACHE:
        meta, per_core, pos_of = _preprocess(np.asarray(inputs["edge_index"]))
        _CACHE["pre"] = (meta, per_core, pos_of)
        _CACHE["prog"] = _build_program(meta, nloc)
    nc = _CACHE["prog"]
    meta, per_core, pos_of = _CACHE["pre"]

    consts = _host_consts(inputs)
    x = np.asarray(inputs["x"], np.float32)
    xp = np.empty_like(x)
    xp[pos_of] = x  # xp[new_pos] = x[old]
    in_maps = []
    for c in range(NCORES):
        xl = np.zeros((nlocp, 128), ml_dtypes.bfloat16)
        xl[:nloc] = xp[c * nloc:(c + 1) * nloc].astype(ml_dtypes.bfloat16)
        in_maps.append({"x_local": xl, **per_core[c], **consts})

    res = run_bass_kernel_spmd(nc, in_maps, core_ids=list(range(NCORES)),
                               trace=trace)
    global _LAST_RES
    _LAST_RES = res
    out = np.concatenate([res.results[c]["out"] for c in range(NCORES)],
                         axis=0)
    out = out[pos_of]  # out_full[old] = out_new[pos_of[old]]
    return out, res.exec_time_ns


def kernel(**inputs):
    return _run(inputs)[0]


# revision 17
# speedup vs baseline: 1.0028x; 1.0028x over previous
"""GATv2 3-layer encoder on 8 Trainium2 NeuronCores (Bass/Tile).

Sharding: nodes split contiguously across 8 cores (graph parallel). Edges
(with self-loops) are owned by the dst node's core; per core they are
grouped by (dst block, src chunk) with per-(block,chunk) runs padded to
128-edge tiles. Per layer: local matmuls (HL|HR = h @ [Wl|Wr]), AllGather
of the HL shard into a full [N,128] table, then an edge phase:
  - hl[src] gathered per (block, chunk) by ONE hardware dma_gather
    (InstDMAGatherAnt, int16 chunk-relative indices, ~5-8 tiles per
    instruction) - amortizes the ~1us fixed SWDGE overhead that dominated
    the per-tile-indirect-DMA baseline. The full-node table is split into
    4 chunks of 25000 rows so indices fit int16.
  - one-hot selector matrices S_e ([edge,dst]) and S_eT ([dst,edge]) are
    precomputed on host (graph constants, reused by all 3 layers) and
    streamed from DRAM per dst block - no on-chip selector builds,
  - tt = G^T + hr^T S_eT accumulated in PSUM: 1 hr matmul per 4-tile
    subgroup + 1 transpose matmul per tile; LeakyReLU on ScalarE (one op
    per subgroup),
  - per-tile score matmul (lhsT = t2t tile, rhs = block-diag attention)
    yields scores in [edge, head] layout; Exp writes exp(score) directly
    into the seg-matmul RHS weight columns (no PSUM->SBUF transposes),
  - per-edge alpha-weighting as one DVE multiply per tile,
  - segment softmax denominator + weighted feature sum via one PE matmul
    per tile (lhsT = S_e), accumulating per 128-dst block in PSUM,
  - per-block epilogue: normalize, bias, relu (f32), fused next-layer
    node-phase matmuls.

kernel(**inputs) takes FULL inputs, returns the FULL [100000, 64] f32 output.
"""

import math
from contextlib import ExitStack

import numpy as np
import ml_dtypes

import concourse.bass as bass
import concourse.tile as tile
from concourse import bacc, mybir
from concourse.bass_utils import run_bass_kernel_spmd

# -------- problem config (hardcoded; must match reference.setup_inputs) ----
N_NODES = 100_000
N_EDGES = 1_600_000
NCORES = 8
NEG_SLOPE = 0.2
P = 128
LAYERS = [(128, 128, 4), (128, 128, 4), (128, 64, 1)]  # (D_in, D_out, heads)
PAD_OFF = 200.0
EPS = 1e-16
GRP = 4       # tiles per matmul/prelu subgroup (PSUM-bank limited)
NCHUNK = 4    # gather-table chunks (int16 index range)
CH_ROWS = N_NODES // NCHUNK

f32 = mybir.dt.float32
bf16 = mybir.dt.bfloat16
i16 = mybir.dt.int16

_CACHE = {}
_LAST_RES = None


# ---------------------------------------------------------------------------
# host-side preprocessing
# ---------------------------------------------------------------------------
def _balance_perm(dst, n, nloc, nblk):
    """Assign nodes to (core, block) bins balancing per-bin edge counts.
    Returns pos_of[old_id] -> new global position."""
    import heapq
    deg = np.bincount(dst, minlength=n).astype(np.int64)
    caps = np.zeros((NCORES, nblk), np.int64)
    caps[:, :] = P
    last = nloc - (nblk - 1) * P
    caps[:, nblk - 1] = last
    order = np.argsort(-deg, kind="stable")
    heap = [(0, c * nblk + b) for c in range(NCORES) for b in range(nblk)]
    heapq.heapify(heap)
    fill = np.zeros(NCORES * nblk, np.int64)
    pos_of = np.empty(n, np.int64)
    for v in order:
        while True:
            w, bin_ = heapq.heappop(heap)
            c, b = divmod(bin_, nblk)
            if fill[bin_] < caps[c, b]:
                break
        s = fill[bin_]
        fill[bin_] += 1
        pos_of[v] = c * nloc + b * P + s
        if fill[bin_] < caps[c, b]:
            heapq.heappush(heap, (w + deg[v], bin_))
    return pos_of


def _preprocess(edge_index):
    n = N_NODES
    nloc = n // NCORES
    nblk = math.ceil(nloc / P)

    src = np.concatenate([edge_index[0], np.arange(n, dtype=np.int64)]).astype(np.int64)
    dst = np.concatenate([edge_index[1], np.arange(n, dtype=np.int64)]).astype(np.int64)

    pos_of = _balance_perm(dst, n, nloc, nblk)
    src = pos_of[src]
    dst = pos_of[dst]

    core_of = dst // nloc
    blk_of = (dst % nloc) // P
    ch_of = src // CH_ROWS
    counts4 = np.zeros((NCORES, nblk, NCHUNK), np.int64)
    np.add.at(counts4, (core_of, blk_of, ch_of), 1)
    tiles_bch = np.ceil(counts4.max(axis=0) / P).astype(np.int64)  # [nblk, 4]
    tbp = tiles_bch.sum(axis=1)                                    # [nblk]
    t_total = int(tbp.sum())
    gcol = np.concatenate([[0], np.cumsum(tbp)])[:-1]
    ch_toff = np.concatenate(
        [np.zeros((nblk, 1), np.int64), np.cumsum(tiles_bch, axis=1)],
        axis=1)[:, :NCHUNK]

    jar = np.arange(P, dtype=np.float32)
    per_core = []
    for c in range(NCORES):
        eids = np.where(core_of == c)[0]
        s_c = src[eids]
        d_c = dst[eids] - c * nloc
        b_c = d_c // P
        off_c = (d_c % P).astype(np.float32)
        chc = s_c // CH_ROWS
        order = np.lexsort((off_c, chc, b_c))
        s_c, off_c, chc, b_c = s_c[order], off_c[order], chc[order], b_c[order]

        key = b_c * NCHUNK + chc
        starts = np.searchsorted(key, np.arange(nblk * NCHUNK))
        ends = np.searchsorted(key, np.arange(nblk * NCHUNK) + 1)
        flat_idx = np.zeros(t_total * P, np.int16)
        dstoff = np.full((t_total, P), PAD_OFF, np.float32)
        for b in range(nblk):
            for ch in range(NCHUNK):
                e0, e1 = starts[b * NCHUNK + ch], ends[b * NCHUNK + ch]
                cnt = e1 - e0
                if cnt == 0:
                    continue
                flat0 = (gcol[b] + ch_toff[b, ch]) * P
                flat_idx[flat0:flat0 + cnt] = \
                    (s_c[e0:e1] - ch * CH_ROWS).astype(np.int16)
                dstoff.reshape(-1)[flat0:flat0 + cnt] = off_c[e0:e1]
        # idx layout for dma_gather: entry i at [i%16, i//16], tiled to 128
        idx16 = np.tile(np.ascontiguousarray(flat_idx.reshape(-1, 16).T),
                        (8, 1))

        # one-hot selectors: per block [SeT tiles | Se tiles] (bf16)
        se_all = (dstoff[:, :, None] == jar[None, None, :])
        se_flat = np.ascontiguousarray(
            se_all.transpose(1, 0, 2).reshape(P, t_total * P))
        seT_flat = np.ascontiguousarray(
            se_all.transpose(2, 0, 1).reshape(P, t_total * P))
        sel = np.empty((P, t_total * 2 * P), np.float32)
        for b in range(nblk):
            c0 = int(gcol[b])
            tb = int(tbp[b])
            base = c0 * 2 * P
            sel[:, base:base + tb * P] = seT_flat[:, c0 * P:(c0 + tb) * P]
            sel[:, base + tb * P:base + 2 * tb * P] = \
                se_flat[:, c0 * P:(c0 + tb) * P]

        per_core.append({
            "idx16": idx16,                            # [128, t_total*8] i16
            "sel": sel.astype(ml_dtypes.bfloat16),     # [128, t_total*256]
        })
    meta = (tiles_bch, tbp, gcol, ch_toff, t_total)
    return meta, per_core, pos_of


def _host_consts(inputs):
    c = {}
    for li, (din, dout, h) in enumerate(LAYERS):
        wl = np.asarray(inputs[f"W{li}l"], np.float32)
        wr = np.asarray(inputs[f"W{li}r"], np.float32)
        att = np.asarray(inputs[f"a{li}"], np.float32)
        bias = np.asarray(inputs[f"b{li}"], np.float32)
        ch = dout // h
        a_bd = np.zeros((dout, h), np.float32)
        for hh in range(h):
            a_bd[hh * ch:(hh + 1) * ch, hh] = att[hh]
        c[f"w2_{li}"] = np.concatenate([wl, wr], axis=1).astype(ml_dtypes.bfloat16)
        c[f"abd_{li}"] = a_bd.astype(ml_dtypes.bfloat16)
        c[f"bias_{li}"] = np.ascontiguousarray(np.tile(bias[None, :], (P, 1)))
    c["ident"] = np.eye(P, dtype=ml_dtypes.bfloat16)
    return c


# ---------------------------------------------------------------------------
# device program
# ---------------------------------------------------------------------------
def _build_program(meta, nloc):
    tiles_bch, tbp, gcol, ch_toff, t_total = meta
    nblk = len(tbp)
    tbpmax = int(tbp.max())
    nlocp = nblk * P
    n = nloc * NCORES

    nc = bacc.Bacc("TRN2", target_bir_lowering=False, debug=False,
                   num_devices=NCORES, dynamic_dma_scratch_size=32768,
                   num_swdge_queues=4)

    x_local = nc.dram_tensor("x_local", [nlocp, 128], bf16, kind="ExternalInput")
    idx_in = nc.dram_tensor("idx16", [P, t_total * 8], i16, kind="ExternalInput")
    sel_in = nc.dram_tensor("sel", [P, t_total * 2 * P], bf16,
                            kind="ExternalInput")
    ident_in = nc.dram_tensor("ident", [P, P], bf16, kind="ExternalInput")
    w2_in, abd_in, bias_in = {}, {}, {}
    for li, (din, dout, h) in enumerate(LAYERS):
        w2_in[li] = nc.dram_tensor(f"w2_{li}", [din, 2 * dout], bf16,
                                   kind="ExternalInput")
        abd_in[li] = nc.dram_tensor(f"abd_{li}", [dout, h], bf16,
                                    kind="ExternalInput")
        bias_in[li] = nc.dram_tensor(f"bias_{li}", [P, dout], f32,
                                     kind="ExternalInput")
    out_t = nc.dram_tensor("out", [nloc, 64], f32, kind="ExternalOutput")

    with tile.TileContext(nc) as tc, ExitStack() as ctx:
        cn = ctx.enter_context(tc.tile_pool(name="cn", bufs=1))
        dr = ctx.enter_context(tc.tile_pool(name="dr", bufs=1, space="DRAM"))

        hr_buf = {0: dr.tile([nlocp + P, 128], bf16, tag="hr0", name="hr0")}
        hr_buf[1] = dr.tile([nlocp + P, 128], bf16, tag="hr1", name="hr1")
        hr_buf[2] = dr.tile([nlocp + P, 64], bf16, tag="hr2", name="hr2")
        # bounce/hlf tables are 128 cols for every layer (layer-2 cols 64:
        # are zero) so the gather elem_size is always 128 (256B rows)
        bounce = {li: dr.tile([nloc, 128], bf16, tag=f"bnc{li}",
                              name=f"bnc{li}") for li in range(3)}
        hlf = {li: dr.tile([n, 128], bf16, addr_space="Shared",
                           tag=f"hlf{li}", name=f"hlf{li}") for li in range(3)}

        ident = cn.tile([P, P], bf16)
        nc.sync.dma_start(out=ident[:], in_=ident_in[:, :])
        idx16 = cn.tile([P, t_total * 8], i16)
        nc.sync.dma_start(out=idx16[:], in_=idx_in[:, :])
        w2_sb, abd_sb, bias_sb = {}, {}, {}
        for li, (din, dout, h) in enumerate(LAYERS):
            w2_sb[li] = cn.tile([din, 2 * dout], bf16, tag=f"w2s{li}",
                                name=f"w2s{li}")
            nc.sync.dma_start(out=w2_sb[li][:], in_=w2_in[li][:, :])
            abd_sb[li] = cn.tile([dout, h], bf16, tag=f"abds{li}",
                                 name=f"abds{li}")
            nc.sync.dma_start(out=abd_sb[li][:], in_=abd_in[li][:, :])
            bias_sb[li] = cn.tile([P, dout], f32, tag=f"biass{li}",
                                  name=f"biass{li}")
            nc.sync.dma_start(out=bias_sb[li][:], in_=bias_in[li][:, :])

        zpad = cn.tile([P, 128], bf16)
        nc.vector.memset(zpad[:], 0.0)
        for r0 in range(nloc, nlocp + P, P):
            rows = min(P, nlocp + P - r0)
            nc.sync.dma_start(out=hr_buf[0][r0:r0 + rows, :], in_=zpad[:rows, :])
            nc.sync.dma_start(out=hr_buf[1][r0:r0 + rows, :], in_=zpad[:rows, :])
            nc.sync.dma_start(out=hr_buf[2][r0:r0 + rows, :64],
                              in_=zpad[:rows, :64])
        # layer-2 bounce cols 64:128 stay zero forever
        for r0 in range(0, nloc, P):
            rows = min(P, nloc - r0)
            nc.sync.dma_start(out=bounce[2][r0:r0 + rows, 64:128],
                              in_=zpad[:rows, :64])

        # ================= layers =================
        ett = ctx.enter_context(tc.tile_pool(name="ett", bufs=3, space="PSUM"))
        esc = ctx.enter_context(tc.tile_pool(name="esc", bufs=2, space="PSUM"))
        enp = ctx.enter_context(tc.tile_pool(name="enp", bufs=1, space="PSUM"))
        eu = ctx.enter_context(tc.tile_pool(name="eu", bufs=2, space="PSUM"))
        esb = ctx.enter_context(tc.tile_pool(name="esb", bufs=3))
        blkp = ctx.enter_context(tc.tile_pool(name="blkp", bufs=4))
        nsb = ctx.enter_context(tc.tile_pool(name="nsb", bufs=4))
        for li, (din, dout, h) in enumerate(LAYERS):
            ch_ = dout // h
            dh = dout + h

            # ---- node phase (standalone for layer 0 only; later layers are
            # fused into the previous edge phase epilogue) ----
            if li == 0:
                for nt in range(nblk):
                    r0 = nt * P
                    rows = min(P, nloc - r0)
                    x_sb = nsb.tile([P, din], bf16, tag="x")
                    nc.sync.dma_start(out=x_sb[:], in_=x_local[r0:r0 + P, :din])
                    xT_ps = enp.tile([P, P], f32, tag="nps")
                    nc.tensor.matmul(out=xT_ps[:din, :P], lhsT=x_sb[:],
                                     rhs=ident[:], start=True, stop=True)
                    xT = nsb.tile([P, P], bf16, tag="xTs")
                    nc.vector.tensor_copy(out=xT[:din, :], in_=xT_ps[:din, :P])
                    hlr_ps = enp.tile([P, 2 * dout], f32, tag="nps",
                                      name="hlrps")
                    nc.tensor.matmul(out=hlr_ps[:], lhsT=xT[:din, :],
                                     rhs=w2_sb[li][:], start=True, stop=True)
                    hl_sb = nsb.tile([P, dout], bf16, tag="hl")
                    nc.scalar.activation(out=hl_sb[:], in_=hlr_ps[:, 0:dout],
                                         func=mybir.ActivationFunctionType.Copy)
                    hr_sb = nsb.tile([P, dout], bf16, tag="hr")
                    nc.scalar.activation(out=hr_sb[:], in_=hlr_ps[:, dout:],
                                         func=mybir.ActivationFunctionType.Copy)
                    nc.sync.dma_start(out=bounce[li][r0:r0 + rows, :dout],
                                      in_=hl_sb[:rows, :])
                    nc.sync.dma_start(out=hr_buf[li][r0:r0 + rows, :dout],
                                      in_=hr_sb[:rows, :])

                nc.gpsimd.collective_compute(
                    "AllGather", mybir.AluOpType.bypass,
                    replica_groups=[list(range(NCORES))],
                    ins=[bounce[0][:].opt()], outs=[hlf[0][:].opt()])

            # ---- edge phase ----
            for b in range(nblk):
                r0 = b * P
                rows = min(P, nloc - r0)
                tb = int(tbp[b])
                c0 = int(gcol[b])
                hrb = blkp.tile([P, dout], bf16, tag="hrb")
                nc.sync.dma_start(out=hrb[:],
                                  in_=hr_buf[li][r0:r0 + P, :dout])
                # selector block: [SeT tiles (tb*P) | Se tiles (tb*P)]
                sel_blk = blkp.tile([P, tbpmax * 2 * P], bf16, tag="sel")
                nc.sync.dma_start(
                    out=sel_blk[:, :tb * 2 * P],
                    in_=sel_in[:, c0 * 2 * P:(c0 + tb) * 2 * P])
                # gathered hl[src] for all tiles of the block, one dma_gather
                # per source chunk (tile-aligned, independent)
                G_blk = blkp.tile([P, tbpmax * P], bf16, tag="G")
                for ch in range(NCHUNK):
                    ntile = int(tiles_bch[b, ch])
                    if ntile == 0:
                        continue
                    tofs = int(ch_toff[b, ch])
                    nidx = ntile * P
                    icol = (c0 + tofs) * 8
                    nc.gpsimd.dma_gather(
                        G_blk[:, (tofs) * P:(tofs + ntile) * P].rearrange(
                            "p (t c) -> p t c", c=P),
                        hlf[li][ch * CH_ROWS:(ch + 1) * CH_ROWS, :],
                        idx16[:, icol:icol + ntile * 8],
                        nidx, nidx, P, queue_num=(b * NCHUNK + ch) % 4)
                u_ps = eu.tile([P, dh], f32, tag="u")

                for t0 in range(0, tb, GRP):
                    k = min(GRP, tb - t0)
                    rhs_blk = esb.tile([P, GRP * dh], bf16, tag="rhs")
                    # tt = hr^T @ SeT (one mm) + per-tile G^T transposes
                    tt_ps = ett.tile([P, GRP * P], f32, tag="tt")
                    nc.tensor.matmul(out=tt_ps[:dout, :k * P],
                                     lhsT=hrb[:, :dout],
                                     rhs=sel_blk[:, t0 * P:(t0 + k) * P],
                                     start=True, stop=False,
                                     skip_group_check=True)
                    for gi in range(k):
                        nc.tensor.matmul(
                            out=tt_ps[:dout, gi * P:(gi + 1) * P],
                            lhsT=G_blk[:, (t0 + gi) * P:(t0 + gi) * P + dout],
                            rhs=ident[:], start=False, stop=True,
                            skip_group_check=True)
                    t2t = esb.tile([P, GRP * P], bf16, tag="t2t")
                    nc.scalar.activation(
                        out=t2t[:dout, :k * P], in_=tt_ps[:dout, :k * P],
                        func=mybir.ActivationFunctionType.Prelu,
                        alpha=NEG_SLOPE)
                    # per-tile scores in [edge, head] layout
                    sc_ps = esc.tile([P, GRP * 4], f32, tag="sc")
                    for gi in range(k):
                        nc.tensor.matmul(
                            out=sc_ps[:, gi * h:(gi + 1) * h],
                            lhsT=t2t[:dout, gi * P:(gi + 1) * P],
                            rhs=abd_sb[li][:], start=True, stop=True)
                    rview = rhs_blk[:, :k * dh].rearrange(
                        "p (t c) -> p t c", c=dh)
                    # exp(score) into the weight cols of rhs_blk (one op)
                    nc.scalar.activation(
                        out=rview[:, :, dout:dh],
                        in_=sc_ps[:, :k * h].rearrange("p (t h) -> p t h",
                                                       h=h),
                        func=mybir.ActivationFunctionType.Exp)
                    # alpha-weighted features into rhs_blk (one op)
                    nc.vector.tensor_tensor(
                        out=rview[:, :, 0:dout].rearrange(
                            "p t (h c) -> p t h c", h=h),
                        in0=G_blk[:, t0 * P:t0 * P + k * P].rearrange(
                            "p (t c) -> p t c", c=P)[:, :, 0:dout].rearrange(
                            "p t (h c) -> p t h c", h=h),
                        in1=rview[:, :, dout:dh].to_broadcast([P, k, h, ch_]),
                        op=mybir.AluOpType.mult)
                    # per-tile segment sum (features + denominator)
                    for gi in range(k):
                        t = t0 + gi
                        nc.tensor.matmul(
                            out=u_ps[:],
                            lhsT=sel_blk[:, (tb + t) * P:(tb + t + 1) * P],
                            rhs=rhs_blk[:, gi * dh:(gi + 1) * dh],
                            start=(t == 0), stop=(t == tb - 1))

                # ---- block epilogue (f32) ----
                den = esb.tile([P, h], f32, tag="den")
                nc.vector.tensor_scalar(
                    out=den[:], in0=u_ps[:, dout:dh], scalar1=EPS,
                    scalar2=None, op0=mybir.AluOpType.add)
                rden = esb.tile([P, h], f32, tag="rden")
                nc.vector.reciprocal(out=rden[:], in_=den[:])
                o_sb = esb.tile([P, dout], f32, tag="osb")
                if h > 1:
                    nc.vector.tensor_tensor(
                        out=o_sb[:].rearrange("p (h c) -> p h c", h=h),
                        in0=u_ps[:, 0:dout].rearrange("p (h c) -> p h c",
                                                      h=h),
                        in1=rden[:].to_broadcast([P, h, ch_]),
                        op=mybir.AluOpType.mult)
                else:
                    nc.vector.tensor_scalar(
                        out=o_sb[:], in0=u_ps[:, 0:dout],
                        scalar1=rden[:, 0:1], scalar2=None,
                        op0=mybir.AluOpType.mult)
                nc.vector.tensor_tensor(out=o_sb[:], in0=o_sb[:],
                                        in1=bias_sb[li][:, :dout],
                                        op=mybir.AluOpType.add)
                if li < 2:
                    o2_sb = esb.tile([P, dout], bf16, tag="o2sb")
                    nc.scalar.activation(
                        out=o2_sb[:], in_=o_sb[:],
                        func=mybir.ActivationFunctionType.Relu)
                    # fused node phase for layer li+1, block b
                    din2 = LAYERS[li + 1][0]
                    dout2 = LAYERS[li + 1][1]
                    xT_ps = enp.tile([P, P], f32, tag="nps", name="xTf")
                    nc.tensor.matmul(out=xT_ps[:din2, :], lhsT=o2_sb[:],
                                     rhs=ident[:], start=True, stop=True)
                    xTf = nsb.tile([P, P], bf16, tag="xTs", name="xTfs")
                    nc.vector.tensor_copy(out=xTf[:din2, :],
                                          in_=xT_ps[:din2, :])
                    hlr2 = enp.tile([P, 2 * dout2], f32, tag="nps",
                                    name="hlr2")
                    nc.tensor.matmul(out=hlr2[:], lhsT=xTf[:din2, :],
                                     rhs=w2_sb[li + 1][:], start=True,
                                     stop=True)
                    hl2 = nsb.tile([P, dout2], bf16, tag="hl", name="hl2")
                    nc.scalar.activation(
                        out=hl2[:], in_=hlr2[:, 0:dout2],
                        func=mybir.ActivationFunctionType.Copy)
                    hr2s = nsb.tile([P, dout2], bf16, tag="hr", name="hr2s")
                    nc.scalar.activation(
                        out=hr2s[:], in_=hlr2[:, dout2:],
                        func=mybir.ActivationFunctionType.Copy)
                    nc.sync.dma_start(out=bounce[li + 1][r0:r0 + rows, :dout2],
                                      in_=hl2[:rows, :])
                    nc.sync.dma_start(
                        out=hr_buf[li + 1][r0:r0 + rows, :dout2],
                        in_=hr2s[:rows, :])
                else:
                    o2f = esb.tile([P, dout], f32, tag="o2f")
                    nc.scalar.activation(
                        out=o2f[:], in_=o_sb[:],
                        func=mybir.ActivationFunctionType.Relu)
                    nc.sync.dma_start(out=out_t[r0:r0 + rows, :],
                                      in_=o2f[:rows, :])

            if li < 2:
                nc.gpsimd.collective_compute(
                    "AllGather", mybir.AluOpType.bypass,
                    replica_groups=[list(range(NCORES))],
                    ins=[bounce[li + 1][:].opt()],
                    outs=[hlf[li + 1][:].opt()])

    nc.compile()
    return nc


def _run(inputs, trace=False):
    n = N_NODES
    nloc = n // NCORES
    nblk = math.ceil(nloc / P)
    nlocp = nblk * P

    if "prog" not in _CACHE:
        meta, per_core, pos_of = _preprocess(np.asarray(inputs["edge_index"]))
        _CACHE["pre"] = (meta, per_core, pos_of)
        _CACHE["prog"] = _build_program(meta, nloc)
    nc = _CACHE["prog"]
    meta, per_core, pos_of = _CACHE["pre"]

    consts = _host_consts(inputs)
    x = np.asarray(inputs["x"], np.float32)
    xp = np.empty_like(x)
    xp[pos_of] = x  # xp[new_pos] = x[old]
    in_maps = []
    for c in range(NCORES):
        xl = np.zeros((nlocp, 128), ml_dtypes.bfloat16)
        xl[:nloc] = xp[c * nloc:(c + 1) * nloc].astype(ml_dtypes.bfloat16)
        in_maps.append({"x_local": xl, **per_core[c], **consts})

    res = run_bass_kernel_spmd(nc, in_maps, core_ids=list(range(NCORES)),
                               trace=trace)
    global _LAST_RES
    _LAST_RES = res
    out = np.concatenate([res.results[c]["out"] for c in range(NCORES)],
                         axis=0)
    out = out[pos_of]  # out_full[old] = out_new[pos_of[old]]
    return out, res.exec_time_ns


def kernel(**inputs):
    return _run(inputs)[0]


# revision 21
# speedup vs baseline: 1.0338x; 1.0309x over previous
"""GATv2 3-layer encoder on 8 Trainium2 NeuronCores (Bass/Tile).

Sharding: nodes split contiguously across 8 cores (graph parallel). Edges
(with self-loops) are owned by the dst node's core; per core they are
grouped by (dst block, src chunk) with per-(block,chunk) runs padded to
128-edge tiles. Per layer: local matmuls (HL|HR = h @ [Wl|Wr]), AllGather
of the HL shard into a full [N,128] table, then an edge phase:
  - hl[src] gathered per (block, chunk) by ONE hardware dma_gather
    (InstDMAGatherAnt, int16 chunk-relative indices, ~5-8 tiles per
    instruction) - amortizes the ~1us fixed SWDGE overhead that dominated
    the per-tile-indirect-DMA baseline. The full-node table is split into
    4 chunks of 25000 rows so indices fit int16.
  - one-hot selector matrices S_e ([edge,dst]) and S_eT ([dst,edge]) are
    precomputed on host (graph constants, reused by all 3 layers) and
    streamed from DRAM per dst block - no on-chip selector builds,
  - tt = G^T + hr^T S_eT accumulated in PSUM: 1 hr matmul per 4-tile
    subgroup + 1 transpose matmul per tile; LeakyReLU on ScalarE (one op
    per subgroup),
  - per-tile score matmul (lhsT = t2t tile, rhs = block-diag attention)
    yields scores in [edge, head] layout; Exp writes exp(score) directly
    into the seg-matmul RHS weight columns (no PSUM->SBUF transposes),
  - per-edge alpha-weighting as one DVE multiply per tile,
  - segment softmax denominator + weighted feature sum via one PE matmul
    per tile (lhsT = S_e), accumulating per 128-dst block in PSUM,
  - per-block epilogue: normalize, bias, relu (f32), fused next-layer
    node-phase matmuls.

kernel(**inputs) takes FULL inputs, returns the FULL [100000, 64] f32 output.
"""

import math
from contextlib import ExitStack

import numpy as np
import ml_dtypes

import concourse.bass as bass
import concourse.tile as tile
from concourse import bacc, mybir
from concourse.bass_utils import run_bass_kernel_spmd

# -------- problem config (hardcoded; must match reference.setup_inputs) ----
N_NODES = 100_000
N_EDGES = 1_600_000
NCORES = 8
NEG_SLOPE = 0.2
P = 128
LAYERS = [(128, 128, 4), (128, 128, 4), (128, 64, 1)]  # (D_in, D_out, heads)
PAD_OFF = 200.0
EPS = 1e-16
GRP = 4       # tiles per matmul/prelu subgroup (PSUM-bank limited)
NCHUNK = 4    # gather-table chunks (int16 index range)
CH_ROWS = N_NODES // NCHUNK

f32 = mybir.dt.float32
bf16 = mybir.dt.bfloat16
i16 = mybir.dt.int16

_CACHE = {}
_LAST_RES = None


# ---------------------------------------------------------------------------
# host-side preprocessing
# ---------------------------------------------------------------------------
def _balance_perm(dst, n, nloc, nblk):
    """Assign nodes to (core, block) bins balancing per-bin edge counts.
    Returns pos_of[old_id] -> new global position."""
    import heapq
    deg = np.bincount(dst, minlength=n).astype(np.int64)
    caps = np.zeros((NCORES, nblk), np.int64)
    caps[:, :] = P
    last = nloc - (nblk - 1) * P
    caps[:, nblk - 1] = last
    order = np.argsort(-deg, kind="stable")
    heap = [(0, c * nblk + b) for c in range(NCORES) for b in range(nblk)]
    heapq.heapify(heap)
    fill = np.zeros(NCORES * nblk, np.int64)
    pos_of = np.empty(n, np.int64)
    for v in order:
        while True:
            w, bin_ = heapq.heappop(heap)
            c, b = divmod(bin_, nblk)
            if fill[bin_] < caps[c, b]:
                break
        s = fill[bin_]
        fill[bin_] += 1
        pos_of[v] = c * nloc + b * P + s
        if fill[bin_] < caps[c, b]:
            heapq.heappush(heap, (w + deg[v], bin_))
    return pos_of


def _preprocess(edge_index):
    n = N_NODES
    nloc = n // NCORES
    nblk = math.ceil(nloc / P)

    src = np.concatenate([edge_index[0], np.arange(n, dtype=np.int64)]).astype(np.int64)
    dst = np.concatenate([edge_index[1], np.arange(n, dtype=np.int64)]).astype(np.int64)

    pos_of = _balance_perm(dst, n, nloc, nblk)
    src = pos_of[src]
    dst = pos_of[dst]

    core_of = dst // nloc
    blk_of = (dst % nloc) // P
    ch_of = src // CH_ROWS
    counts4 = np.zeros((NCORES, nblk, NCHUNK), np.int64)
    np.add.at(counts4, (core_of, blk_of, ch_of), 1)
    tiles_bch = np.ceil(counts4.max(axis=0) / P).astype(np.int64)  # [nblk, 4]
    tbp = tiles_bch.sum(axis=1)                                    # [nblk]
    t_total = int(tbp.sum())
    gcol = np.concatenate([[0], np.cumsum(tbp)])[:-1]
    ch_toff = np.concatenate(
        [np.zeros((nblk, 1), np.int64), np.cumsum(tiles_bch, axis=1)],
        axis=1)[:, :NCHUNK]

    jar = np.arange(P, dtype=np.float32)
    per_core = []
    for c in range(NCORES):
        eids = np.where(core_of == c)[0]
        s_c = src[eids]
        d_c = dst[eids] - c * nloc
        b_c = d_c // P
        off_c = (d_c % P).astype(np.float32)
        chc = s_c // CH_ROWS
        order = np.lexsort((off_c, chc, b_c))
        s_c, off_c, chc, b_c = s_c[order], off_c[order], chc[order], b_c[order]

        key = b_c * NCHUNK + chc
        starts = np.searchsorted(key, np.arange(nblk * NCHUNK))
        ends = np.searchsorted(key, np.arange(nblk * NCHUNK) + 1)
        flat_idx = np.zeros(t_total * P, np.int16)
        dstoff = np.full((t_total, P), PAD_OFF, np.float32)
        for b in range(nblk):
            for ch in range(NCHUNK):
                e0, e1 = starts[b * NCHUNK + ch], ends[b * NCHUNK + ch]
                cnt = e1 - e0
                if cnt == 0:
                    continue
                flat0 = (gcol[b] + ch_toff[b, ch]) * P
                flat_idx[flat0:flat0 + cnt] = \
                    (s_c[e0:e1] - ch * CH_ROWS).astype(np.int16)
                dstoff.reshape(-1)[flat0:flat0 + cnt] = off_c[e0:e1]
        # idx layout for dma_gather: entry i at [i%16, i//16], tiled to 128
        idx16 = np.tile(np.ascontiguousarray(flat_idx.reshape(-1, 16).T),
                        (8, 1))

        # one-hot selectors: per block [SeT tiles | Se tiles] (bf16)
        se_all = (dstoff[:, :, None] == jar[None, None, :])
        se_flat = np.ascontiguousarray(
            se_all.transpose(1, 0, 2).reshape(P, t_total * P))
        seT_flat = np.ascontiguousarray(
            se_all.transpose(2, 0, 1).reshape(P, t_total * P))
        sel = np.empty((P, t_total * 2 * P), np.float32)
        for b in range(nblk):
            c0 = int(gcol[b])
            tb = int(tbp[b])
            base = c0 * 2 * P
            sel[:, base:base + tb * P] = seT_flat[:, c0 * P:(c0 + tb) * P]
            sel[:, base + tb * P:base + 2 * tb * P] = \
                se_flat[:, c0 * P:(c0 + tb) * P]

        per_core.append({
            "idx16": idx16,                            # [128, t_total*8] i16
            "sel": sel.astype(ml_dtypes.bfloat16),     # [128, t_total*256]
        })
    meta = (tiles_bch, tbp, gcol, ch_toff, t_total)
    return meta, per_core, pos_of


def _host_consts(inputs):
    c = {}
    for li, (din, dout, h) in enumerate(LAYERS):
        wl = np.asarray(inputs[f"W{li}l"], np.float32)
        wr = np.asarray(inputs[f"W{li}r"], np.float32)
        att = np.asarray(inputs[f"a{li}"], np.float32)
        bias = np.asarray(inputs[f"b{li}"], np.float32)
        ch = dout // h
        a_bd = np.zeros((dout, h), np.float32)
        for hh in range(h):
            a_bd[hh * ch:(hh + 1) * ch, hh] = att[hh]
        c[f"w2_{li}"] = np.concatenate([wl, wr], axis=1).astype(ml_dtypes.bfloat16)
        c[f"abd_{li}"] = a_bd.astype(ml_dtypes.bfloat16)
        c[f"bias_{li}"] = np.ascontiguousarray(np.tile(bias[None, :], (P, 1)))
    c["ident"] = np.eye(P, dtype=ml_dtypes.bfloat16)
    return c


# ---------------------------------------------------------------------------
# device program
# ---------------------------------------------------------------------------
def _build_program(meta, nloc):
    tiles_bch, tbp, gcol, ch_toff, t_total = meta
    nblk = len(tbp)
    tbpmax = int(tbp.max())
    nlocp = nblk * P
    n = nloc * NCORES

    nc = bacc.Bacc("TRN2", target_bir_lowering=False, debug=False,
                   num_devices=NCORES, dynamic_dma_scratch_size=32768,
                   num_swdge_queues=4)

    x_local = nc.dram_tensor("x_local", [nlocp, 128], bf16, kind="ExternalInput")
    idx_in = nc.dram_tensor("idx16", [P, t_total * 8], i16, kind="ExternalInput")
    sel_in = nc.dram_tensor("sel", [P, t_total * 2 * P], bf16,
                            kind="ExternalInput")
    ident_in = nc.dram_tensor("ident", [P, P], bf16, kind="ExternalInput")
    w2_in, abd_in, bias_in = {}, {}, {}
    for li, (din, dout, h) in enumerate(LAYERS):
        w2_in[li] = nc.dram_tensor(f"w2_{li}", [din, 2 * dout], bf16,
                                   kind="ExternalInput")
        abd_in[li] = nc.dram_tensor(f"abd_{li}", [dout, h], bf16,
                                    kind="ExternalInput")
        bias_in[li] = nc.dram_tensor(f"bias_{li}", [P, dout], f32,
                                     kind="ExternalInput")
    out_t = nc.dram_tensor("out", [nloc, 64], f32, kind="ExternalOutput")

    with tile.TileContext(nc) as tc, ExitStack() as ctx:
        cn = ctx.enter_context(tc.tile_pool(name="cn", bufs=1))
        dr = ctx.enter_context(tc.tile_pool(name="dr", bufs=1, space="DRAM"))

        hr_buf = {0: dr.tile([nlocp + P, 128], bf16, tag="hr0", name="hr0")}
        hr_buf[1] = dr.tile([nlocp + P, 128], bf16, tag="hr1", name="hr1")
        hr_buf[2] = dr.tile([nlocp + P, 64], bf16, tag="hr2", name="hr2")
        # bounce/hlf tables are 128 cols for every layer (layer-2 cols 64:
        # are zero) so the gather elem_size is always 128 (256B rows)
        bounce = {li: dr.tile([nloc, 128], bf16, tag=f"bnc{li}",
                              name=f"bnc{li}") for li in range(3)}
        hlf = {li: dr.tile([n, 128], bf16, addr_space="Shared",
                           tag=f"hlf{li}", name=f"hlf{li}") for li in range(3)}

        ident = cn.tile([P, P], bf16)
        nc.sync.dma_start(out=ident[:], in_=ident_in[:, :])
        idx16 = cn.tile([P, t_total * 8], i16)
        nc.sync.dma_start(out=idx16[:], in_=idx_in[:, :])
        w2_sb, abd_sb, bias_sb = {}, {}, {}
        for li, (din, dout, h) in enumerate(LAYERS):
            w2_sb[li] = cn.tile([din, 2 * dout], bf16, tag=f"w2s{li}",
                                name=f"w2s{li}")
            nc.sync.dma_start(out=w2_sb[li][:], in_=w2_in[li][:, :])
            abd_sb[li] = cn.tile([dout, h], bf16, tag=f"abds{li}",
                                 name=f"abds{li}")
            nc.sync.dma_start(out=abd_sb[li][:], in_=abd_in[li][:, :])
            bias_sb[li] = cn.tile([P, dout], f32, tag=f"biass{li}",
                                  name=f"biass{li}")
            nc.sync.dma_start(out=bias_sb[li][:], in_=bias_in[li][:, :])

        zpad = cn.tile([P, 128], bf16)
        nc.vector.memset(zpad[:], 0.0)
        for r0 in range(nloc, nlocp + P, P):
            rows = min(P, nlocp + P - r0)
            nc.sync.dma_start(out=hr_buf[0][r0:r0 + rows, :], in_=zpad[:rows, :])
            nc.sync.dma_start(out=hr_buf[1][r0:r0 + rows, :], in_=zpad[:rows, :])
            nc.sync.dma_start(out=hr_buf[2][r0:r0 + rows, :64],
                              in_=zpad[:rows, :64])
        # layer-2 bounce cols 64:128 stay zero forever
        for r0 in range(0, nloc, P):
            rows = min(P, nloc - r0)
            nc.sync.dma_start(out=bounce[2][r0:r0 + rows, 64:128],
                              in_=zpad[:rows, :64])

        # ================= layers =================
        ett = ctx.enter_context(tc.tile_pool(name="ett", bufs=3, space="PSUM"))
        esc = ctx.enter_context(tc.tile_pool(name="esc", bufs=2, space="PSUM"))
        enp = ctx.enter_context(tc.tile_pool(name="enp", bufs=1, space="PSUM"))
        eu = ctx.enter_context(tc.tile_pool(name="eu", bufs=2, space="PSUM"))
        esb = ctx.enter_context(tc.tile_pool(name="esb", bufs=3))
        blkp = ctx.enter_context(tc.tile_pool(name="blkp", bufs=4))
        nsb = ctx.enter_context(tc.tile_pool(name="nsb", bufs=4))
        for li, (din, dout, h) in enumerate(LAYERS):
            ch_ = dout // h
            dh = dout + h

            # ---- node phase (standalone for layer 0 only; later layers are
            # fused into the previous edge phase epilogue) ----
            if li == 0:
                for nt in range(nblk):
                    r0 = nt * P
                    rows = min(P, nloc - r0)
                    x_sb = nsb.tile([P, din], bf16, tag="x")
                    nc.sync.dma_start(out=x_sb[:], in_=x_local[r0:r0 + P, :din])
                    xT_ps = enp.tile([P, P], f32, tag="nps")
                    nc.tensor.matmul(out=xT_ps[:din, :P], lhsT=x_sb[:],
                                     rhs=ident[:], start=True, stop=True)
                    xT = nsb.tile([P, P], bf16, tag="xTs")
                    nc.vector.tensor_copy(out=xT[:din, :], in_=xT_ps[:din, :P])
                    hlr_ps = enp.tile([P, 2 * dout], f32, tag="nps",
                                      name="hlrps")
                    nc.tensor.matmul(out=hlr_ps[:], lhsT=xT[:din, :],
                                     rhs=w2_sb[li][:], start=True, stop=True)
                    hl_sb = nsb.tile([P, dout], bf16, tag="hl")
                    nc.scalar.activation(out=hl_sb[:], in_=hlr_ps[:, 0:dout],
                                         func=mybir.ActivationFunctionType.Copy)
                    hr_sb = nsb.tile([P, dout], bf16, tag="hr")
                    nc.scalar.activation(out=hr_sb[:], in_=hlr_ps[:, dout:],
                                         func=mybir.ActivationFunctionType.Copy)
                    nc.sync.dma_start(out=bounce[li][r0:r0 + rows, :dout],
                                      in_=hl_sb[:rows, :])
                    nc.sync.dma_start(out=hr_buf[li][r0:r0 + rows, :dout],
                                      in_=hr_sb[:rows, :])

                nc.gpsimd.collective_compute(
                    "AllGather", mybir.AluOpType.bypass,
                    replica_groups=[list(range(NCORES))],
                    ins=[bounce[0][:].opt()], outs=[hlf[0][:].opt()])

            # ---- edge phase (software-pipelined emission, skew 2) ----
            sgs = []
            for b in range(nblk):
                for t0 in range(0, int(tbp[b]), GRP):
                    sgs.append((b, t0, min(GRP, int(tbp[b]) - t0)))
            nsg = len(sgs)
            bstate = {}
            states = {}

            def prefetch_block(b, li=li, dout=dout):
                if b >= nblk:
                    return
                tb = int(tbp[b])
                c0 = int(gcol[b])
                r0 = b * P
                hrb = blkp.tile([P, dout], bf16, tag="hrb")
                nc.sync.dma_start(out=hrb[:],
                                  in_=hr_buf[li][r0:r0 + P, :dout])
                sel_blk = blkp.tile([P, tbpmax * 2 * P], bf16, tag="sel")
                nc.sync.dma_start(
                    out=sel_blk[:, :tb * 2 * P],
                    in_=sel_in[:, c0 * 2 * P:(c0 + tb) * 2 * P])
                G_blk = blkp.tile([P, tbpmax * P], bf16, tag="G")
                for cq in range(NCHUNK):
                    ntile = int(tiles_bch[b, cq])
                    if ntile == 0:
                        continue
                    tofs = int(ch_toff[b, cq])
                    nidx = ntile * P
                    icol = (c0 + tofs) * 8
                    nc.gpsimd.dma_gather(
                        G_blk[:, tofs * P:(tofs + ntile) * P].rearrange(
                            "p (t c) -> p t c", c=P),
                        hlf[li][cq * CH_ROWS:(cq + 1) * CH_ROWS, :],
                        idx16[:, icol:icol + ntile * 8],
                        nidx, nidx, P, queue_num=(b * NCHUNK + cq) % 4)
                bstate[b] = {"hrb": hrb, "sel": sel_blk, "G": G_blk,
                             "u": None}

            def s1(i, li=li, dout=dout):
                b, t0, k = sgs[i]
                if i == 0 or sgs[i - 1][0] != b:
                    # entering block b: keep 2 blocks prefetched ahead
                    if b + 2 < nblk and (b + 2) not in bstate:
                        prefetch_block(b + 2)
                bs = bstate[b]
                tt_ps = ett.tile([P, GRP * P], f32, tag="tt")
                nc.tensor.matmul(out=tt_ps[:dout, :k * P],
                                 lhsT=bs["hrb"][:, :dout],
                                 rhs=bs["sel"][:, t0 * P:(t0 + k) * P],
                                 start=True, stop=False,
                                 skip_group_check=True)
                for gi in range(k):
                    nc.tensor.matmul(
                        out=tt_ps[:dout, gi * P:(gi + 1) * P],
                        lhsT=bs["G"][:, (t0 + gi) * P:(t0 + gi) * P + dout],
                        rhs=ident[:], start=False, stop=True,
                        skip_group_check=True)
                t2t = esb.tile([P, GRP * P], bf16, tag="t2t")
                nc.scalar.activation(
                    out=t2t[:dout, :k * P], in_=tt_ps[:dout, :k * P],
                    func=mybir.ActivationFunctionType.Prelu,
                    alpha=NEG_SLOPE)
                states[i] = {"t2t": t2t}

            def s2(i, li=li, dout=dout, h=h, ch_=ch_, dh=dh):
                b, t0, k = sgs[i]
                st = states[i]
                t2t = st["t2t"]
                sc_ps = esc.tile([P, GRP * 4], f32, tag="sc")
                for gi in range(k):
                    nc.tensor.matmul(
                        out=sc_ps[:, gi * h:(gi + 1) * h],
                        lhsT=t2t[:dout, gi * P:(gi + 1) * P],
                        rhs=abd_sb[li][:], start=True, stop=True)
                rhs_blk = esb.tile([P, GRP * dh], bf16, tag="rhs")
                rview = rhs_blk[:, :k * dh].rearrange("p (t c) -> p t c",
                                                      c=dh)
                nc.scalar.activation(
                    out=rview[:, :, dout:dh],
                    in_=sc_ps[:, :k * h].rearrange("p (t h) -> p t h", h=h),
                    func=mybir.ActivationFunctionType.Exp)
                nc.vector.tensor_tensor(
                    out=rview[:, :, 0:dout].rearrange(
                        "p t (h c) -> p t h c", h=h),
                    in0=bstate[b]["G"][:, t0 * P:t0 * P + k * P].rearrange(
                        "p (t c) -> p t c", c=P)[:, :, 0:dout].rearrange(
                        "p t (h c) -> p t h c", h=h),
                    in1=rview[:, :, dout:dh].to_broadcast([P, k, h, ch_]),
                    op=mybir.AluOpType.mult)
                st["rhs"] = rhs_blk

            def epilogue(b, u_ps, li=li, dout=dout, h=h, ch_=ch_, dh=dh):
                r0 = b * P
                rows = min(P, nloc - r0)
                den = esb.tile([P, h], f32, tag="den")
                nc.vector.tensor_scalar(
                    out=den[:], in0=u_ps[:, dout:dh], scalar1=EPS,
                    scalar2=None, op0=mybir.AluOpType.add)
                rden = esb.tile([P, h], f32, tag="rden")
                nc.vector.reciprocal(out=rden[:], in_=den[:])
                o_sb = esb.tile([P, dout], f32, tag="osb")
                if h > 1:
                    nc.vector.tensor_tensor(
                        out=o_sb[:].rearrange("p (h c) -> p h c", h=h),
                        in0=u_ps[:, 0:dout].rearrange("p (h c) -> p h c",
                                                      h=h),
                        in1=rden[:].to_broadcast([P, h, ch_]),
                        op=mybir.AluOpType.mult)
                else:
                    nc.vector.tensor_scalar(
                        out=o_sb[:], in0=u_ps[:, 0:dout],
                        scalar1=rden[:, 0:1], scalar2=None,
                        op0=mybir.AluOpType.mult)
                nc.vector.tensor_tensor(out=o_sb[:], in0=o_sb[:],
                                        in1=bias_sb[li][:, :dout],
                                        op=mybir.AluOpType.add)
                if li < 2:
                    o2_sb = esb.tile([P, dout], bf16, tag="o2sb")
                    nc.scalar.activation(
                        out=o2_sb[:], in_=o_sb[:],
                        func=mybir.ActivationFunctionType.Relu)
                    din2 = LAYERS[li + 1][0]
                    dout2 = LAYERS[li + 1][1]
                    xT_ps = enp.tile([P, P], f32, tag="nps", name="xTf")
                    nc.tensor.matmul(out=xT_ps[:din2, :], lhsT=o2_sb[:],
                                     rhs=ident[:], start=True, stop=True)
                    xTf = nsb.tile([P, P], bf16, tag="xTs", name="xTfs")
                    nc.vector.tensor_copy(out=xTf[:din2, :],
                                          in_=xT_ps[:din2, :])
                    hlr2 = enp.tile([P, 2 * dout2], f32, tag="nps",
                                    name="hlr2")
                    nc.tensor.matmul(out=hlr2[:], lhsT=xTf[:din2, :],
                                     rhs=w2_sb[li + 1][:], start=True,
                                     stop=True)
                    hl2 = nsb.tile([P, dout2], bf16, tag="hl", name="hl2")
                    nc.scalar.activation(
                        out=hl2[:], in_=hlr2[:, 0:dout2],
                        func=mybir.ActivationFunctionType.Copy)
                    hr2s = nsb.tile([P, dout2], bf16, tag="hr", name="hr2s")
                    nc.scalar.activation(
                        out=hr2s[:], in_=hlr2[:, dout2:],
                        func=mybir.ActivationFunctionType.Copy)
                    nc.sync.dma_start(
                        out=bounce[li + 1][r0:r0 + rows, :dout2],
                        in_=hl2[:rows, :])
                    nc.sync.dma_start(
                        out=hr_buf[li + 1][r0:r0 + rows, :dout2],
                        in_=hr2s[:rows, :])
                else:
                    o2f = esb.tile([P, dout], f32, tag="o2f")
                    nc.scalar.activation(
                        out=o2f[:], in_=o_sb[:],
                        func=mybir.ActivationFunctionType.Relu)
                    nc.sync.dma_start(out=out_t[r0:r0 + rows, :],
                                      in_=o2f[:rows, :])

            def s3(i, dh=dh):
                b, t0, k = sgs[i]
                tb = int(tbp[b])
                bs = bstate[b]
                if bs["u"] is None:
                    bs["u"] = eu.tile([P, dh], f32, tag="u", name="u_ps")
                rhs_blk = states[i]["rhs"]
                for gi in range(k):
                    t = t0 + gi
                    nc.tensor.matmul(
                        out=bs["u"][:],
                        lhsT=bs["sel"][:, (tb + t) * P:(tb + t + 1) * P],
                        rhs=rhs_blk[:, gi * dh:(gi + 1) * dh],
                        start=(t == 0), stop=(t == tb - 1))
                if t0 + k == tb:
                    epilogue(b, bs["u"])
                    del bstate[b]
                del states[i]

            for bb in range(min(3, nblk)):
                prefetch_block(bb)
            for i in range(nsg + 2):
                if i < nsg:
                    s1(i)
                if 1 <= i <= nsg:
                    s2(i - 1)
                if i >= 2:
                    s3(i - 2)

            if li < 2:
                nc.gpsimd.collective_compute(
                    "AllGather", mybir.AluOpType.bypass,
                    replica_groups=[list(range(NCORES))],
                    ins=[bounce[li + 1][:].opt()],
                    outs=[hlf[li + 1][:].opt()])

    nc.compile()
    return nc


def _run(inputs, trace=False):
    n = N_NODES
    nloc = n // NCORES
    nblk = math.ceil(nloc / P)
    nlocp = nblk * P

    if "prog" not in _CACHE:
        meta, per_core, pos_of = _preprocess(np.asarray(inputs["edge_index"]))
        _CACHE["pre"] = (meta, per_core, pos_of)
        _CACHE["prog"] = _build_program(meta, nloc)
    nc = _CACHE["prog"]
    meta, per_core, pos_of = _CACHE["pre"]

    consts = _host_consts(inputs)
    x = np.asarray(inputs["x"], np.float32)
    xp = np.empty_like(x)
    xp[pos_of] = x  # xp[new_pos] = x[old]
    in_maps = []
    for c in range(NCORES):
        xl = np.zeros((nlocp, 128), ml_dtypes.bfloat16)
        xl[:nloc] = xp[c * nloc:(c + 1) * nloc].astype(ml_dtypes.bfloat16)
        in_maps.append({"x_local": xl, **per_core[c], **consts})

    res = run_bass_kernel_spmd(nc, in_maps, core_ids=list(range(NCORES)),
                               trace=trace)
    global _LAST_RES
    _LAST_RES = res
    out = np.concatenate([res.results[c]["out"] for c in range(NCORES)],
                         axis=0)
    out = out[pos_of]  # out_full[old] = out_new[pos_of[old]]
    return out, res.exec_time_ns


def kernel(**inputs):
    return _run(inputs)[0]
